# revision 7
# baseline (speedup 1.0000x reference)
"""HebbNet forward on 8 NeuronCores (Bass/Tile), data-parallel over batch.

Math: the fast-weight matrix A is internal state only. With
A_t = eta * sum_{s<t} lam^{t-1-s} h_s x_s^T, the term (A_t @ x_t) collapses
via the per-batch Gram matrix G[s,t] = x_s . x_t to
    a1fast[t] = sum_{s<t} eta * lam^{t-1-s} * G[s,t] * h_s,
so the T-step recurrence runs in the T-dim "kernel space" (coefficients on
h vectors) and A is never materialized. Verified vs the reference: ~3e-6
max rel err in fp32 (same as the reference's own fp32-vs-fp64 error).

Per-core layout (B_local = 4, rows r = 4*t + b, 128 rows = 128 partitions):
  x_rows  [128, 512]  input rows
  U_rows  [128, 512]  x @ w1.T + b1 (PE, via PE-transposed x and w1)
  G       [128, 128]  row Gram = X X^T (PE)
  coeffs  [128, 128]  G * mask, mask[r=(s,b'),j=(t,b)] = eta*lam^(t-1-s)*[b'=b][s<=t-3]

Compute engines require 32-aligned partition bases, so the per-step h
(4 rows at base 4t) cannot be written into H_rows by ScalarE directly.
Instead each step's sigmoid lands in a base-0 staging tile; a DMA (which
may write any partitions) scatters it into H_rows. The matmul mask only
covers s <= t-3, giving the DMA ~3 steps of slack; the s=t-1 / s=t-2
terms enter via two small PE matmuls whose lhsT are per-step diagonal
coefficient tiles D1[:,t,:], D2[:,t,:] built in the preamble from
shifted-x dot products (g1 = x_{t-1}.x_t, g2 = x_{t-2}.x_t).

  step t: ps[4,512] = I128[:,4t:4t+4].T @ U_rows        (row select)
                    + coeffs[:,4t:4t+4].T @ H_rows      (s <= t-3)
                    + D1[:,t,:].T @ stage_{t-1}         (s = t-1)
                    + D2[:,t,:].T @ stage_{t-2}         (s = t-2)
          stage_t = sigmoid(ps)   (ScalarE, base 0)
          DMA H_rows[4t:4t+4] <- stage_t
  y: per 32-row chunk, PE-transpose H chunk, y = sigmoid(H @ w2 + b2).
Matmuls with N>=256 run as float32r (1 cyc/row vs fp32's 4); fp32r
consumers require fp32r-typed producers, so those tiles are F32R.
"""

import json
import os

import numpy as np

import concourse.bass as bass
import concourse.bass2jax as bass2jax
import concourse.mybir as mybir
import concourse.tile as tile
from concourse import bass_utils
from concourse.bass_utils import run_bass_kernel_spmd
from concourse.tile import ScopedClock


def _split_waits(bir_bytes, max_waits=1):
    """Cap sync waits per instruction; spill extras onto same-engine NoOps.

    Walrus CoreV2/V3 codegen rejects instructions carrying more than a
    couple of sync-wait commands ("Too many sync wait commands"). Tile's
    sem assignment freely attaches several waits to one instruction, so
    rewrite the BIR: each excess wait moves to a dedicated NoOp emitted
    just before the instruction on the same engine (program order makes
    this equivalent).
    """
    bir = json.loads(bir_bytes)
    n = 0
    for fn in bir["functions"]:
        for bb in fn["blocks"]:
            insts = bb.get("instructions")
            if not insts:
                continue
            out = []
            for inst in insts:
                si = inst.get("sync_info")
                waits = (si or {}).get("on_wait") or []
                if len(waits) > max_waits:
                    for w in waits[:-max_waits]:
                        n += 1
                        nop = {
                            "engine": inst["engine"],
                            "ins": [],
                            "outs": [],
                            "name": f"I-wsplit-{n}",
                            "opcode": "NoOp",
                            "sync_info": {"on_update": [], "on_wait": [w]},
                        }
                        if "debug" in inst:
                            nop["debug"] = inst["debug"]
                        out.append(nop)
                    si["on_wait"] = waits[-max_waits:]
                out.append(inst)
            bb["instructions"] = out
    return json.dumps(bir).encode()


if not getattr(bass_utils, "_hebb_wait_split", False):
    _orig_compile_bir_kernel = bass_utils.compile_bir_kernel

    def _compile_bir_kernel_split(bir_json, tmpdir, neff_name="file.neff"):
        return _orig_compile_bir_kernel(_split_waits(bir_json), tmpdir, neff_name)

    bass_utils.compile_bir_kernel = _compile_bir_kernel_split
    bass2jax.compile_bir_kernel = _compile_bir_kernel_split
    bass_utils._hebb_wait_split = True

T, B, NX, NH, NY = 32, 32, 512, 512, 64
NCORES = 8
BL = B // NCORES  # batches per core
R = T * BL  # 128 rows per core

F32 = mybir.dt.float32
USE_F32R = os.environ.get("HEBB_F32R", "1") == "1"
DTR = mybir.dt.float32r if USE_F32R else F32
SIG = mybir.ActivationFunctionType.Sigmoid


class _TileContext(tile.TileContext):
    """TileContext whose kernel-tail drain carries one sem wait per SP nop.

    The stock _drain_and_barrier puts every live semaphore's final-value
    wait on a single SP Drain; walrus CoreV3 codegen rejects CTRL
    instructions with more than a couple of sync waits ("Too many sync
    wait commands"). Spread the waits across dedicated nops instead.
    """

    def _drain_and_barrier(self, tick_clock, wait_clock):
        nc = self.nc
        drain_inst = nc.sync.drain()
        wait_clock.add_sem_waits(
            drain_inst.ins, ScopedClock({None: tick_clock.global_clock})
        )
        si = drain_inst.ins.sync_info
        waits = list(si.on_wait) if si is not None else []
        if len(waits) > 1:
            drain_inst.ins.sync_info = mybir.SyncInfo(
                on_wait=[], on_update=list(si.on_update)
            )
            for w in waits:
                nop = nc.sync.nop()
                nop.ins.sync_info = mybir.SyncInfo(on_wait=[w], on_update=[])

        nc.all_engine_barrier()
        assert self.sems is not None
        popped = nc._tile_sem_poison_stack.pop()
        assert popped is self._sem_poison
        nc.clear_and_free_semaphores(list(self.sems.allocated().values()))
        nc.all_engine_barrier()


def build():
    nc = bass.Bass(target_bir_lowering=False)

    x_d = nc.dram_tensor("x_local", [R, NX], F32, kind="ExternalInput")
    w1_d = nc.dram_tensor("w1", [NH, NX], F32, kind="ExternalInput")
    b1_d = nc.dram_tensor("b1r", [1, NH], DTR, kind="ExternalInput")
    w2_d = nc.dram_tensor("w2", [NH, NY], F32, kind="ExternalInput")
    b2_d = nc.dram_tensor("b2r", [1, NY], DTR, kind="ExternalInput")
    mask_d = nc.dram_tensor("masks", [R, R], F32, kind="ExternalInput")
    id_d = nc.dram_tensor("ident", [128, 128], F32, kind="ExternalInput")
    idr_d = nc.dram_tensor("identr", [128, 128], DTR, kind="ExternalInput")
    p1_d = nc.dram_tensor("p4a", [BL, T, BL], F32, kind="ExternalInput")
    p2_d = nc.dram_tensor("p4b", [BL, T, BL], F32, kind="ExternalInput")
    h_out = nc.dram_tensor("h_out", [R, NH], DTR, kind="ExternalOutput")
    y_out = nc.dram_tensor("y_out", [R, NY], F32, kind="ExternalOutput")

    NCH = NX // 128  # 4 contraction chunks

    with _TileContext(nc) as tc:
        with (
            tc.tile_pool(name="sb", bufs=1) as sb,
            tc.tile_pool(name="stg", bufs=4) as stg,
            tc.tile_pool(name="pre", bufs=2, space="PSUM") as pre,
            tc.tile_pool(name="ps", bufs=4, space="PSUM") as psp,
            tc.tile_pool(name="post", bufs=2, space="PSUM") as post,
            tc.tile_pool(name="dram", bufs=1, space="DRAM") as dram,
        ):
            # Warm the ACT sigmoid table while DMAs run.
            warm = sb.tile([1, 1], F32)
            nc.vector.memset(warm[:], 0.0)
            nc.scalar.activation(out=warm[:], in_=warm[:], func=SIG)

            x_rows = sb.tile([R, NX], F32)
            w1_sb = sb.tile([128, NCH, NX], F32)
            w2_sb = sb.tile([128, NCH, NY], F32)
            b1_row = sb.tile([1, NH], DTR)
            b2_row = sb.tile([1, NY], DTR)
            masks = sb.tile([R, R], F32)
            ident = sb.tile([128, 128], F32)
            identr = sb.tile([128, 128], DTR)
            ones = sb.tile([1, 128], DTR)
            p4a = sb.tile([BL, T, BL], F32)
            p4b = sb.tile([BL, T, BL], F32)

            nc.sync.dma_start(x_rows[:], x_d[:])
            nc.sync.dma_start(w1_sb[:], w1_d[:].rearrange("(c p) x -> p c x", p=128))
            nc.sync.dma_start(w2_sb[:], w2_d[:].rearrange("(c p) n -> p c n", p=128))
            nc.sync.dma_start(b1_row[:], b1_d[:])
            nc.sync.dma_start(b2_row[:], b2_d[:])
            nc.sync.dma_start(masks[:], mask_d[:])
            nc.sync.dma_start(ident[:], id_d[:])
            nc.sync.dma_start(identr[:], idr_d[:])
            nc.sync.dma_start(p4a[:], p1_d[:])
            nc.sync.dma_start(p4b[:], p2_d[:])
            ones32 = sb.tile([1, 128], F32)
            nc.vector.memset(ones32[:], 1.0)
            nc.vector.tensor_copy(ones[:], ones32[:])

            # Shifted copies of x (rows r-4, r-8) for adjacent-step Gram terms.
            x_s4 = sb.tile([R, NX], F32)
            x_s8 = sb.tile([R, NX], F32)
            nc.vector.memset(x_s4[:], 0.0)
            nc.vector.memset(x_s8[:], 0.0)
            nc.sync.dma_start(x_s4[4:R, :], x_d[0 : R - 4, :])
            nc.sync.dma_start(x_s8[8:R, :], x_d[0 : R - 8, :])

            # g1[r=(t,b)] = x_{t-1,b} . x_{t,b}; g2: x_{t-2,b} . x_{t,b}
            prod = sb.tile([R, NX], F32, tag="prod")
            g1 = sb.tile([R, 1], F32)
            g2 = sb.tile([R, 1], F32)
            nc.vector.tensor_mul(prod[:], x_rows[:], x_s4[:])
            nc.vector.tensor_reduce(
                g1[:], prod[:], axis=mybir.AxisListType.X, op=mybir.AluOpType.add
            )
            prod2 = sb.tile([R, NX], F32, tag="prod")
            nc.vector.tensor_mul(prod2[:], x_rows[:], x_s8[:])
            nc.vector.tensor_reduce(
                g2[:], prod2[:], axis=mybir.AxisListType.X, op=mybir.AluOpType.add
            )

            # Reshape g[(t,b)] -> g_bt[b, t] via a DRAM bounce (partition moves
            # are DMA-only).
            g_dram = dram.tile([2, R], F32)
            nc.sync.dma_start(g_dram[0, :], g1[:, 0])
            nc.sync.dma_start(g_dram[1, :], g2[:, 0])
            g1_bt = sb.tile([BL, T], F32)
            g2_bt = sb.tile([BL, T], F32)
            nc.sync.dma_start(g1_bt[:], g_dram[0, :].rearrange("(t b) -> b t", b=BL))
            nc.sync.dma_start(g2_bt[:], g_dram[1, :].rearrange("(t b) -> b t", b=BL))

            # D1[b,t,b'] = delta(b,b') * eta * g1_bt[b,t]  (p4a carries
            # delta*eta, zeroed at t=0); D2 likewise with eta*lam, zero t<2.
            g1_rep = sb.tile([BL, T, BL], F32)
            g2_rep = sb.tile([BL, T, BL], F32)
            for i in range(BL):
                nc.vector.tensor_copy(g1_rep[:, :, i], g1_bt[:])
                nc.vector.tensor_copy(g2_rep[:, :, i], g2_bt[:])
            D1 = sb.tile([BL, T, BL], DTR)
            D2 = sb.tile([BL, T, BL], DTR)
            nc.vector.tensor_mul(D1[:], p4a[:], g1_rep[:])
            nc.vector.tensor_mul(D2[:], p4b[:], g2_rep[:])

            # x^T chunks: xT[p, c, r] = x_rows[r, 128c+p]
            xT = sb.tile([128, NCH, R], DTR)
            xt_ps = pre.tile([128, NCH * 128], F32, tag="pre")
            for c in range(NCH):
                nc.tensor.transpose(
                    xt_ps[:, bass.ts(c, 128)], x_rows[:, bass.ts(c, 128)], ident[:]
                )
            nc.vector.tensor_copy(xT[:].rearrange("p c r -> p (c r)"), xt_ps[:])

            # Gram of rows: G[r, j] = x_r . x_j
            g_ps = pre.tile([R, R], F32, tag="pre")
            for c in range(NCH):
                nc.tensor.matmul(
                    g_ps[:],
                    xT[:, c, :],
                    xT[:, c, :],
                    start=(c == 0),
                    stop=(c == NCH - 1),
                )
            coeffs = sb.tile([R, R], DTR)
            nc.vector.tensor_mul(coeffs[:], g_ps[:], masks[:])

            # w1^T chunks: w1T[p, cx, nh] = w1[nh, 128*cx + p]
            w1T = sb.tile([128, NCH, NH], DTR)
            for cx in range(NCH):
                wt_ps = pre.tile([128, NH], F32, tag="pre")
                for cn in range(NCH):
                    nc.tensor.transpose(
                        wt_ps[:, bass.ts(cn, 128)],
                        w1_sb[:, cn, bass.ts(cx, 128)],
                        ident[:],
                    )
                nc.vector.tensor_copy(w1T[:, cx, :], wt_ps[:])

            # U = x @ w1^T + b1 (broadcast over rows via ones-outer matmul)
            u_ps = pre.tile([R, NH], F32, tag="pre")
            for c in range(NCH):
                nc.tensor.matmul(
                    u_ps[:], xT[:, c, :], w1T[:, c, :], start=(c == 0), stop=False
                )
            nc.tensor.matmul(u_ps[:], ones[:], b1_row[:], start=False, stop=True)
            U_rows = sb.tile([R, NH], DTR)
            nc.vector.tensor_copy(U_rows[:], u_ps[:])

            H_rows = sb.tile([R, NH], DTR)
            zsrc = sb.tile([R, NH], F32)
            nc.vector.memset(zsrc[:], 0.0)
            nc.vector.tensor_copy(H_rows[:], zsrc[:])
            y_rows = sb.tile([R, NY], F32)

            stages = []
            for t in range(T):
                ps = psp.tile([BL, NH], F32, tag="step")
                n_mm = 1 + (t >= 3) + (t >= 1) + (t >= 2)
                nc.tensor.matmul(
                    ps[:],
                    identr[:, bass.ts(t, BL)],
                    U_rows[:],
                    start=True,
                    stop=(n_mm == 1),
                )
                left = n_mm - 1
                if t >= 3:
                    nc.tensor.matmul(
                        ps[:],
                        coeffs[:, bass.ts(t, BL)],
                        H_rows[:],
                        start=False,
                        stop=(left == 1),
                    )
                    left -= 1
                if t >= 1:
                    nc.tensor.matmul(
                        ps[:],
                        D1[:, t, :],
                        stages[t - 1][:],
                        start=False,
                        stop=(left == 1),
                    )
                    left -= 1
                if t >= 2:
                    nc.tensor.matmul(
                        ps[:],
                        D2[:, t, :],
                        stages[t - 2][:],
                        start=False,
                        stop=True,
                    )

                stage = stg.tile([BL, NH], DTR, tag="stage")
                nc.scalar.activation(out=stage[:], in_=ps[:], func=SIG)
                stages.append(stage)
                nc.sync.dma_start(H_rows[bass.ts(t, BL), :], stage[:])

                # After each 8-step block, fold the finished 32 rows into y
                # and stream both outputs out while the recurrence continues.
                if t % 8 == 7:
                    k = t // 8
                    rs = bass.ts(k, 32)
                    tp = (96, 0) if k == 3 else None
                    ht_ps = post.tile([128, 128], DTR, tag="post")
                    for c in range(NCH):
                        nc.tensor.transpose(
                            ht_ps[:, bass.ts(c, 32)],
                            H_rows[rs, bass.ts(c, 128)],
                            identr[bass.ts(k, 32), bass.ts(k, 32)],
                            tile_position=tp,
                        )
                    htc = sb.tile([128, NCH, 32], F32, tag="htc")
                    nc.vector.tensor_copy(
                        htc[:].rearrange("p c r -> p (c r)"), ht_ps[:]
                    )
                    y_ps = post.tile([32, NY], F32, tag="post")
                    for c in range(NCH):
                        nc.tensor.matmul(
                            y_ps[:],
                            htc[:, c, :],
                            w2_sb[:, c, :],
                            start=(c == 0),
                            stop=False,
                        )
                    nc.tensor.matmul(
                        y_ps[:], ones[:, :32], b2_row[:], start=False, stop=True
                    )
                    nc.scalar.activation(out=y_rows[rs, :], in_=y_ps[:], func=SIG)
                    nc.gpsimd.dma_start(h_out[rs, :], H_rows[rs, :])
                    nc.gpsimd.dma_start(y_out[rs, :], y_rows[rs, :])

    return nc


_built = None


def _get_built():
    global _built
    if _built is None:
        _built = build()
    return _built


def kernel(x, w1, b1, w2, b2, lam, eta):
    x = np.ascontiguousarray(np.asarray(x, dtype=np.float32))
    w1 = np.ascontiguousarray(np.asarray(w1, dtype=np.float32))
    b1 = np.asarray(b1, dtype=np.float32).reshape(1, NH)
    w2 = np.ascontiguousarray(np.asarray(w2, dtype=np.float32))
    b2 = np.asarray(b2, dtype=np.float32).reshape(1, NY)
    lam_c = min(float(np.asarray(lam)), 1.0)
    eta_f = float(np.asarray(eta))

    # mask[r=(s,b'), j=(t,b)] = eta * lam^(t-1-s) * [b'==b] * [s <= t-3]
    s_idx = np.arange(T)
    pw = np.zeros((T, T), dtype=np.float64)
    for t in range(3, T):
        pw[t, : t - 2] = eta_f * lam_c ** (t - 1 - s_idx[: t - 2])
    mask = np.zeros((R, R), dtype=np.float32)
    for bb in range(BL):
        mask[bb::BL, bb::BL] = pw.T.astype(np.float32)
    ident = np.eye(128, dtype=np.float32)

    # Diagonal patterns for the s=t-1 / s=t-2 terms.
    p4a = np.zeros((BL, T, BL), dtype=np.float32)
    p4b = np.zeros((BL, T, BL), dtype=np.float32)
    for bb in range(BL):
        p4a[bb, 1:, bb] = eta_f
        p4b[bb, 2:, bb] = eta_f * lam_c

    nc = _get_built()
    shared = {
        "w1": w1,
        "b1r": b1,
        "w2": w2,
        "b2r": b2,
        "masks": mask,
        "ident": ident,
        "identr": ident,
        "p4a": p4a,
        "p4b": p4b,
    }
    in_maps = []
    for c in range(NCORES):
        xl = np.ascontiguousarray(x[:, c * BL : (c + 1) * BL, :]).reshape(R, NX)
        in_maps.append({"x_local": xl, **shared})

    trace = os.environ.get("HEBB_TRACE", "0") == "1"
    res = run_bass_kernel_spmd(nc, in_maps, core_ids=list(range(NCORES)), trace=trace)
    if trace:
        kernel.last_result = res

    hiddens = np.empty((T, B, NH), dtype=np.float32)
    outputs = np.empty((T, B, NY), dtype=np.float32)
    for c in range(NCORES):
        hiddens[:, c * BL : (c + 1) * BL, :] = res.results[c]["h_out"].reshape(
            T, BL, NH
        )
        outputs[:, c * BL : (c + 1) * BL, :] = res.results[c]["y_out"].reshape(
            T, BL, NY
        )
    return hiddens[..., None], outputs


# revision 8
# speedup vs baseline: 1.6215x; 1.6215x over previous
"""HebbNet forward on 8 NeuronCores (Bass/Tile), data-parallel over batch.

Math: the fast-weight matrix A is internal state only. With
A_t = eta * sum_{s<t} lam^{t-1-s} h_s x_s^T, the term (A_t @ x_t) collapses
via the per-batch Gram matrix G[s,t] = x_s . x_t to
    a1fast[t] = sum_{s<t} eta * lam^{t-1-s} * G[s,t] * h_s,
so the T-step recurrence runs in the T-dim "kernel space" (coefficients on
h vectors) and A is never materialized. Verified vs the reference: ~3e-6
max rel err in fp32 (same as the reference's own fp32-vs-fp64 error).

Per-core layout (B_local = 4, rows r = 4*t + b, 128 rows = 128 partitions):
  x_rows  [128, 512]  input rows
  U_rows  [128, 512]  x @ w1.T + b1 (PE, via PE-transposed x and w1)
  G       [128, 128]  row Gram = X X^T (PE)
  coeffs  [128, 128]  G * mask, mask[r=(s,b'),j=(t,b)] = eta*lam^(t-1-s)*[b'=b][s<=t-2]

Compute engines require 32-aligned partition bases, so the per-step h
(4 rows at base 4t) cannot be written into H_rows by ScalarE directly.
Each step's sigmoid lands in a base-0 staging tile; a DMA (arbitrary
partitions allowed) scatters it into H_rows. The s=t-1 term reads the
staging tile directly via a small PE matmul whose lhsT is a per-step
diagonal coefficient tile D1[:,t,:] built in the preamble from the
shifted-x dot product g1[(t,b)] = x_{t-1,b} . x_{t,b}. The coeff matmul
contracts only rows [0 : 4(t-1)] so its dependency is on the scatter from
two steps back, keeping the ~1us DMA latency off the critical path.

  step t: ps[4,512] = I128[:,4t:4t+4].T @ U_rows          (row select)
                    + coeffs[0:4t-4, 4t:4t+4].T @ H_rows[0:4t-4]
                    + D1[:,t,:].T @ stage_{t-1}            (s = t-1)
          stage_t = sigmoid(ps)   (ScalarE, base 0)
          DMA H_rows[4t:4t+4] <- stage_t                   (GpSimd queue)
  y: per 32-row chunk, PE-transpose H chunk, y = sigmoid(H @ w2 + b2).
Matmuls with N>=256 run as float32r (1 cyc/row vs fp32's 4); fp32r
consumers require fp32r-typed producers, so those tiles are F32R.
"""

import json
import os

import numpy as np

import concourse.bass as bass
import concourse.bass2jax as bass2jax
import concourse.mybir as mybir
import concourse.tile as tile
from concourse import bass_utils
from concourse.bass_utils import run_bass_kernel_spmd
from concourse.tile import ScopedClock


def _split_waits(bir_bytes, max_waits=1):
    """Cap sync waits per instruction; spill extras onto same-engine NoOps.

    Walrus CoreV2/V3 codegen rejects instructions carrying more than a
    couple of sync-wait commands ("Too many sync wait commands"). Tile's
    sem assignment freely attaches several waits to one instruction, so
    rewrite the BIR: each excess wait moves to a dedicated NoOp emitted
    just before the instruction on the same engine (program order makes
    this equivalent).
    """
    bir = json.loads(bir_bytes)
    n = 0
    for fn in bir["functions"]:
        for bb in fn["blocks"]:
            insts = bb.get("instructions")
            if not insts:
                continue
            out = []
            for inst in insts:
                si = inst.get("sync_info")
                waits = (si or {}).get("on_wait") or []
                if len(waits) > max_waits:
                    for w in waits[:-max_waits]:
                        n += 1
                        nop = {
                            "engine": inst["engine"],
                            "ins": [],
                            "outs": [],
                            "name": f"I-wsplit-{n}",
                            "opcode": "NoOp",
                            "sync_info": {"on_update": [], "on_wait": [w]},
                        }
                        if "debug" in inst:
                            nop["debug"] = inst["debug"]
                        out.append(nop)
                    si["on_wait"] = waits[-max_waits:]
                out.append(inst)
            bb["instructions"] = out
    return json.dumps(bir).encode()


if not getattr(bass_utils, "_hebb_wait_split", False):
    _orig_compile_bir_kernel = bass_utils.compile_bir_kernel

    def _compile_bir_kernel_split(bir_json, tmpdir, neff_name="file.neff"):
        return _orig_compile_bir_kernel(_split_waits(bir_json), tmpdir, neff_name)

    bass_utils.compile_bir_kernel = _compile_bir_kernel_split
    bass2jax.compile_bir_kernel = _compile_bir_kernel_split
    bass_utils._hebb_wait_split = True


T, B, NX, NH, NY = 32, 32, 512, 512, 64
NCORES = 8
BL = B // NCORES  # batches per core
R = T * BL  # 128 rows per core

F32 = mybir.dt.float32
USE_F32R = os.environ.get("HEBB_F32R", "1") == "1"
DTR = mybir.dt.float32r if USE_F32R else F32
SIG = mybir.ActivationFunctionType.Sigmoid


class _TileContext(tile.TileContext):
    """TileContext whose kernel-tail drain carries one sem wait per SP nop.

    The stock _drain_and_barrier puts every live semaphore's final-value
    wait on a single SP Drain; walrus rejects CTRL instructions with more
    than a couple of sync waits. Spread the waits across dedicated nops.
    """

    def _drain_and_barrier(self, tick_clock, wait_clock):
        nc = self.nc
        drain_inst = nc.sync.drain()
        wait_clock.add_sem_waits(
            drain_inst.ins, ScopedClock({None: tick_clock.global_clock})
        )
        si = drain_inst.ins.sync_info
        waits = list(si.on_wait) if si is not None else []
        if len(waits) > 1:
            drain_inst.ins.sync_info = mybir.SyncInfo(
                on_wait=[], on_update=list(si.on_update)
            )
            for w in waits:
                nop = nc.sync.nop()
                nop.ins.sync_info = mybir.SyncInfo(on_wait=[w], on_update=[])

        nc.all_engine_barrier()
        assert self.sems is not None
        popped = nc._tile_sem_poison_stack.pop()
        assert popped is self._sem_poison
        nc.clear_and_free_semaphores(list(self.sems.allocated().values()))
        nc.all_engine_barrier()


def build():
    nc = bass.Bass(target_bir_lowering=False)

    x_d = nc.dram_tensor("x_local", [R, NX], F32, kind="ExternalInput")
    w1_d = nc.dram_tensor("w1", [NH, NX], F32, kind="ExternalInput")
    w2_d = nc.dram_tensor("w2", [NH, NY], F32, kind="ExternalInput")
    # consts packed [128, 384]: cols 0:128 masks, 128:256 ident, 256:384 identr
    cst_d = nc.dram_tensor("consts", [128, 384], F32, kind="ExternalInput")
    # rows packed [BL, 704]: [0,0:512] b1, [0,512:576] b2, [:,576:704] p4a
    row_d = nc.dram_tensor("rowpack", [BL, 704], F32, kind="ExternalInput")
    h_out = nc.dram_tensor("h_out", [R, NH], DTR, kind="ExternalOutput")
    y_out = nc.dram_tensor("y_out", [R, NY], F32, kind="ExternalOutput")

    NCH = NX // 128  # 4 contraction chunks

    with _TileContext(nc) as tc:
        with (
            tc.tile_pool(name="sb", bufs=1) as sb,
            tc.tile_pool(name="stg", bufs=4) as stg,
            tc.tile_pool(name="pre", bufs=2, space="PSUM") as pre,
            tc.tile_pool(name="ps", bufs=4, space="PSUM") as psp,
            tc.tile_pool(name="post", bufs=2, space="PSUM") as post,
            tc.tile_pool(name="dram", bufs=1, space="DRAM") as dram,
        ):
            # Warm the ACT sigmoid table while DMAs run.
            warm = sb.tile([1, 1], F32)
            nc.vector.memset(warm[:], 0.0)
            nc.scalar.activation(out=warm[:], in_=warm[:], func=SIG)

            x_rows = sb.tile([R, NX], F32)
            x_s4 = sb.tile([R, NX], F32)
            w1_sb = sb.tile([128, NCH, NX], F32)
            w2_sb = sb.tile([128, NCH, NY], F32)
            consts = sb.tile([128, 3, 128], F32)
            rowpack = sb.tile([BL, 704], F32)
            b1_row = sb.tile([1, NH], DTR)
            b2_row = sb.tile([1, NY], DTR)
            ones = sb.tile([1, 128], DTR)

            nc.sync.dma_start(x_rows[:], x_d[:])
            nc.sync.dma_start(w1_sb[:], w1_d[:].rearrange("(c p) x -> p c x", p=128))
            nc.scalar.dma_start(w2_sb[:], w2_d[:].rearrange("(c p) n -> p c n", p=128))
            nc.scalar.dma_start(consts[:].rearrange("p c x -> p (c x)"), cst_d[:])
            nc.scalar.dma_start(rowpack[:], row_d[:])
            nc.vector.memset(x_s4[:], 0.0)
            nc.sync.dma_start(x_s4[4:R, :], x_d[0 : R - 4, :])

            masks = consts[:, 0, :]
            ident = consts[:, 1, :]
            identr_f32 = consts[:, 2, :]
            identr = sb.tile([128, 128], DTR)
            nc.vector.tensor_copy(identr[:], identr_f32)
            nc.vector.tensor_copy(b1_row[:], rowpack[0:1, 0:512])
            nc.vector.tensor_copy(b2_row[:], rowpack[0:1, 512:576])
            ones32 = sb.tile([1, 128], F32)
            nc.vector.memset(ones32[:], 1.0)
            nc.vector.tensor_copy(ones[:], ones32[:])

            # g1[r=(t,b)] = x_{t-1,b} . x_{t,b}
            prod = sb.tile([R, NX], F32)
            g1 = sb.tile([R, 1], F32)
            nc.vector.tensor_mul(prod[:], x_rows[:], x_s4[:])
            nc.vector.tensor_reduce(
                g1[:], prod[:], axis=mybir.AxisListType.X, op=mybir.AluOpType.add
            )
            # Reshape g1[(t,b)] -> g1_bt[b, t] via a DRAM bounce (partition
            # moves are DMA-only), replicate, and scale by the p4a pattern
            # (delta(b,b') * eta, zero at t=0) into the diag tiles D1.
            g_dram = dram.tile([R], F32)
            nc.sync.dma_start(g_dram[:], g1[:, 0])
            g1_bt = sb.tile([BL, T], F32)
            nc.sync.dma_start(g1_bt[:], g_dram[:].rearrange("(t b) -> b t", b=BL))
            g1_rep = sb.tile([BL, T, BL], F32)
            for i in range(BL):
                nc.vector.tensor_copy(g1_rep[:, :, i], g1_bt[:])
            D1 = sb.tile([BL, T, BL], DTR)
            nc.vector.tensor_mul(
                D1[:], rowpack[:, 576:704].rearrange("b (t c) -> b t c", c=BL), g1_rep[:]
            )

            # x^T chunks: xT[p, c, r] = x_rows[r, 128c+p]
            xT = sb.tile([128, NCH, R], DTR)
            xt_ps = pre.tile([128, NCH * 128], F32, tag="pre")
            for c in range(NCH):
                nc.tensor.transpose(
                    xt_ps[:, bass.ts(c, 128)], x_rows[:, bass.ts(c, 128)], ident
                )
            nc.vector.tensor_copy(xT[:].rearrange("p c r -> p (c r)"), xt_ps[:])

            # Gram of rows: G[r, j] = x_r . x_j
            g_ps = pre.tile([R, R], F32, tag="pre")
            for c in range(NCH):
                nc.tensor.matmul(
                    g_ps[:],
                    xT[:, c, :],
                    xT[:, c, :],
                    start=(c == 0),
                    stop=(c == NCH - 1),
                )
            coeffs = sb.tile([R, R], DTR)
            nc.vector.tensor_mul(coeffs[:], g_ps[:], masks)

            # w1^T chunks: w1T[p, cx, nh] = w1[nh, 128*cx + p]
            w1T = sb.tile([128, NCH, NH], DTR)
            for cx in range(NCH):
                wt_ps = pre.tile([128, NH], F32, tag="pre")
                for cn in range(NCH):
                    nc.tensor.transpose(
                        wt_ps[:, bass.ts(cn, 128)],
                        w1_sb[:, cn, bass.ts(cx, 128)],
                        ident,
                    )
                nc.vector.tensor_copy(w1T[:, cx, :], wt_ps[:])

            # U = x @ w1^T + b1 (broadcast over rows via ones-outer matmul)
            u_ps = pre.tile([R, NH], F32, tag="pre")
            for c in range(NCH):
                nc.tensor.matmul(
                    u_ps[:], xT[:, c, :], w1T[:, c, :], start=(c == 0), stop=False
                )
            nc.tensor.matmul(u_ps[:], ones[:], b1_row[:], start=False, stop=True)
            U_rows = sb.tile([R, NH], DTR)
            nc.vector.tensor_copy(U_rows[:], u_ps[:])

            H_rows = sb.tile([R, NH], DTR)
            zsrc = sb.tile([R, NH], F32)
            nc.vector.memset(zsrc[:], 0.0)
            nc.vector.tensor_copy(H_rows[:], zsrc[:])
            y_rows = sb.tile([R, NY], F32)

            stages = []
            for t in range(T):
                ps = psp.tile([BL, NH], F32, tag="step")
                n_mm = 1 + (t >= 2) + (t >= 1)
                nc.tensor.matmul(
                    ps[:],
                    identr[:, bass.ts(t, BL)],
                    U_rows[:],
                    start=True,
                    stop=(n_mm == 1),
                )
                if t >= 2:
                    k = 4 * (t - 1)
                    nc.tensor.matmul(
                        ps[:],
                        coeffs[0:k, bass.ts(t, BL)],
                        H_rows[0:k, :],
                        start=False,
                        stop=False,
                    )
                if t >= 1:
                    nc.tensor.matmul(
                        ps[:],
                        D1[:, t, :],
                        stages[t - 1][:],
                        start=False,
                        stop=True,
                    )

                stage = stg.tile([BL, NH], DTR, tag="stage")
                nc.scalar.activation(out=stage[:], in_=ps[:], func=SIG)
                stages.append(stage)
                nc.gpsimd.dma_start(H_rows[bass.ts(t, BL), :], stage[:])

                # After each 8-step block, fold the finished 32 rows into y
                # and stream both outputs out while the recurrence continues.
                if t % 8 == 7:
                    k = t // 8
                    rs = bass.ts(k, 32)
                    tp = (96, 0) if k == 3 else None
                    ht_ps = post.tile([128, 128], DTR, tag="post")
                    for c in range(NCH):
                        nc.tensor.transpose(
                            ht_ps[:, bass.ts(c, 32)],
                            H_rows[rs, bass.ts(c, 128)],
                            identr[bass.ts(k, 32), bass.ts(k, 32)],
                            tile_position=tp,
                        )
                    htc = sb.tile([128, NCH, 32], F32, tag="htc")
                    nc.vector.tensor_copy(
                        htc[:].rearrange("p c r -> p (c r)"), ht_ps[:]
                    )
                    y_ps = post.tile([32, NY], F32, tag="post")
                    for c in range(NCH):
                        nc.tensor.matmul(
                            y_ps[:],
                            htc[:, c, :],
                            w2_sb[:, c, :],
                            start=(c == 0),
                            stop=False,
                        )
                    nc.tensor.matmul(
                        y_ps[:], ones[:, :32], b2_row[:], start=False, stop=True
                    )
                    nc.scalar.activation(out=y_rows[rs, :], in_=y_ps[:], func=SIG)
                    nc.gpsimd.dma_start(h_out[rs, :], H_rows[rs, :])
                    nc.gpsimd.dma_start(y_out[rs, :], y_rows[rs, :])

    return nc


_built = None


def _get_built():
    global _built
    if _built is None:
        _built = build()
    return _built


def kernel(x, w1, b1, w2, b2, lam, eta):
    x = np.ascontiguousarray(np.asarray(x, dtype=np.float32))
    w1 = np.ascontiguousarray(np.asarray(w1, dtype=np.float32))
    b1 = np.asarray(b1, dtype=np.float32).reshape(NH)
    w2 = np.ascontiguousarray(np.asarray(w2, dtype=np.float32))
    b2 = np.asarray(b2, dtype=np.float32).reshape(NY)
    lam_c = min(float(np.asarray(lam)), 1.0)
    eta_f = float(np.asarray(eta))

    # mask[r=(s,b'), j=(t,b)] = eta * lam^(t-1-s) * [b'==b] * [s <= t-2]
    s_idx = np.arange(T)
    pw = np.zeros((T, T), dtype=np.float64)
    for t in range(2, T):
        pw[t, : t - 1] = eta_f * lam_c ** (t - 1 - s_idx[: t - 1])
    mask = np.zeros((R, R), dtype=np.float32)
    for bb in range(BL):
        mask[bb::BL, bb::BL] = pw.T.astype(np.float32)
    ident = np.eye(128, dtype=np.float32)
    consts = np.concatenate([mask, ident, ident], axis=1)

    # rowpack: b1 | b2 | p4a (diag pattern delta(b,b')*eta, zero at t=0)
    p4a = np.zeros((BL, T, BL), dtype=np.float32)
    for bb in range(BL):
        p4a[bb, 1:, bb] = eta_f
    rowpack = np.zeros((BL, 704), dtype=np.float32)
    rowpack[0, 0:512] = b1
    rowpack[0, 512:576] = b2
    rowpack[:, 576:704] = p4a.reshape(BL, T * BL)

    nc = _get_built()
    shared = {"w1": w1, "w2": w2, "consts": consts, "rowpack": rowpack}
    in_maps = []
    for c in range(NCORES):
        xl = np.ascontiguousarray(x[:, c * BL : (c + 1) * BL, :]).reshape(R, NX)
        in_maps.append({"x_local": xl, **shared})

    trace = os.environ.get("HEBB_TRACE", "0") == "1"
    res = run_bass_kernel_spmd(nc, in_maps, core_ids=list(range(NCORES)), trace=trace)
    if trace:
        kernel.last_result = res

    hiddens = np.empty((T, B, NH), dtype=np.float32)
    outputs = np.empty((T, B, NY), dtype=np.float32)
    for c in range(NCORES):
        hiddens[:, c * BL : (c + 1) * BL, :] = res.results[c]["h_out"].reshape(
            T, BL, NH
        )
        outputs[:, c * BL : (c + 1) * BL, :] = res.results[c]["y_out"].reshape(
            T, BL, NY
        )
    return hiddens[..., None], outputs
